# revision 4
# baseline (speedup 1.0000x reference)
"""Lookahead depthwise convolution on 8 Trainium2 NeuronCores.

out[t, b, f] = sum_{c=0..80} x[t+c, b, f] * weight[f, c], zero-padded at the
right edge. x: (2048, 32, 1280) fp32, weight: (1280, 81) fp32.

Feature-sharded across 8 cores (160 features each; the conv is depthwise so
features are fully independent). Per feature the time conv is a banded
Toeplitz matmul: with 128-wide time tiles,
  out_i = A_f^T x_i + B_f^T x_{i+1}
  A_f[m, p] = w[f, m - p]        (0 <= m - p <= 80)   [128 x 128]
  B_f[m, p] = w[f, m + 128 - p]  (0 <= m+128-p <= 80) [80 x 128, rows m<80]

Key layout trick: the host transposes x to feature-major panels
x_panel[f, m, i*32 + b] = x[i*128 + m, b, f] so each feature's ENTIRE time
series is one contiguous [128, 512] fp16 tile. One weight load then streams
512 columns (all 16 time blocks x 32 batch) instead of 32 — the stationary
band is loaded once per feature instead of once per (feature, time block),
keeping the PE streaming-bound instead of LDWEIGHTS-bound. The B band
accumulates the next-block contribution into the same PSUM bank at a
32-column offset; its contraction is trimmed to the 80 nonzero rows.

All device I/O is fp16 (inputs rounded on host, outputs upcast on host);
fp32 PSUM accumulation keeps rel err ~5e-4, well under the 2e-2 gate. The
kernel is HBM-read-stream bound: 21 MB x + 8.3 MB bands per core in, 21 MB
out per core out, with reads and writes on largely independent paths.

Features are processed in groups of 16 so every x/out DMA is a 1-2 MB
contiguous transfer (>90% DMA efficiency). PSUM->SBUF eviction alternates
between the Vector and Scalar engines; in-DMAs ride the SP HWDGE ring while
out-DMAs ride the Activation ring. Pipeline fill and drain are trimmed:
group 0's band/x loads are quarter-sliced so the first matmul starts after
~1/4 of the group's bytes, and the last group's out-DMA is quarter-sliced
so it trails the final eviction by a quarter group instead of half.
"""

import numpy as np

import concourse.bass as bass
import concourse.bacc as bacc
import concourse.mybir as mybir
from concourse import tile
from concourse.bass_utils import run_bass_kernel_spmd

S, B, F, K = 2048, 32, 1280, 81
N_CORES = 8
FC = F // N_CORES      # features per core (160)
NB = S // 128          # time blocks (16)
G = 16                 # features per DMA group
NG = FC // G           # groups per core (10)
XW = G * NB * B        # x free width per group tile (8192)
BW = G * 128           # band free width per group tile (2048)
H = XW // 2            # half-group x width (4096)
Q = XW // 4            # quarter-group x width (2048)

_compiled = None


def _build_program(repeat=1):
    nc = bacc.Bacc("TRN2", target_bir_lowering=False, debug=False)
    f32, f16 = mybir.dt.float32, mybir.dt.float16

    x_in = nc.declare_dram_parameter("x", [NG * 128, XW], f16, isOutput=False)
    a_in = nc.declare_dram_parameter("bandA", [NG * 128, BW], f16,
                                     isOutput=False)
    b_in = nc.declare_dram_parameter("bandB", [NG * 80, BW], f16,
                                     isOutput=False)
    out_ext = nc.declare_dram_parameter("out", [NG * 128, XW], f16,
                                        isOutput=True)

    n_iter = NG * repeat
    with tile.TileContext(nc) as tc:
        with (
            tc.tile_pool(name="xg", bufs=3) as xpool,
            tc.tile_pool(name="ag", bufs=3) as apool,
            tc.tile_pool(name="bg", bufs=3) as bpool,
            tc.tile_pool(name="og", bufs=3) as opool,
            tc.tile_pool(name="ps", bufs=6, space="PSUM") as ppool,
        ):
            for it in range(n_iter):
                g = it % NG
                first, last = it == 0, it == n_iter - 1
                arows = a_in[g * 128:(g + 1) * 128, :]
                brows = b_in[g * 80:(g + 1) * 80, :]
                xrows = x_in[g * 128:(g + 1) * 128, :]
                ag = apool.tile([128, BW], f16)
                bg = bpool.tile([80, BW], f16)
                xg = xpool.tile([128, XW], f16)
                if first:
                    # quarter-sliced so matmul j=0 waits on ~1/4 of the bytes
                    bq = BW // 4
                    for q in range(4):
                        nc.sync.dma_start(out=ag[:, q * bq:(q + 1) * bq],
                                          in_=arows[:, q * bq:(q + 1) * bq])
                        nc.sync.dma_start(out=bg[:, q * bq:(q + 1) * bq],
                                          in_=brows[:, q * bq:(q + 1) * bq])
                        nc.sync.dma_start(out=xg[:, q * Q:(q + 1) * Q],
                                          in_=xrows[:, q * Q:(q + 1) * Q])
                else:
                    nc.sync.dma_start(out=ag[:], in_=arows)
                    nc.sync.dma_start(out=bg[:], in_=brows)
                    nc.sync.dma_start(out=xg[:, 0:H], in_=xrows[:, 0:H])
                    nc.sync.dma_start(out=xg[:, H:XW], in_=xrows[:, H:XW])
                og = opool.tile([128, XW], f16)
                for j in range(G):
                    ps = ppool.tile([128, 512], f32)
                    nc.tensor.matmul(
                        out=ps[:],
                        lhsT=ag[:, j * 128:(j + 1) * 128],
                        rhs=xg[:, j * 512:(j + 1) * 512],
                        start=True, stop=False)
                    nc.tensor.matmul(
                        out=ps[:, 0:480],
                        lhsT=bg[:, j * 128:(j + 1) * 128],
                        rhs=xg[0:80, j * 512 + 32:(j + 1) * 512],
                        start=False, stop=True)
                    dst = og[:, j * 512:(j + 1) * 512]
                    if j % 2 == 0:
                        nc.vector.tensor_copy(out=dst, in_=ps[:])
                    else:
                        nc.scalar.copy(out=dst, in_=ps[:])
                    if last:
                        if j % 4 == 3:
                            q = j // 4
                            nc.scalar.dma_start(
                                out=out_ext[g * 128:(g + 1) * 128,
                                            q * Q:(q + 1) * Q],
                                in_=og[:, q * Q:(q + 1) * Q])
                    elif j == G // 2 - 1:
                        nc.scalar.dma_start(
                            out=out_ext[g * 128:(g + 1) * 128, 0:H],
                            in_=og[:, 0:H])
                if not last:
                    nc.scalar.dma_start(
                        out=out_ext[g * 128:(g + 1) * 128, H:XW],
                        in_=og[:, H:XW])
    nc.finalize()
    return nc


def _build_bands(weight):
    w16 = weight.astype(np.float16)
    m = np.arange(128)[:, None]
    p = np.arange(128)[None, :]
    dA = m - p
    mA = (dA >= 0) & (dA < K)
    iA = np.clip(dA, 0, K - 1)
    mr = np.arange(80)[:, None]
    dB = mr + 128 - p
    mB = (dB >= 0) & (dB < K)
    iB = np.clip(dB, 0, K - 1)

    A = np.where(mA[None], w16[:, iA], np.float16(0))   # [F, 128, 128]
    Bm = np.where(mB[None], w16[:, iB], np.float16(0))  # [F, 80, 128]
    A = A.reshape(N_CORES, NG, G, 128, 128).transpose(0, 1, 3, 2, 4)
    A = np.ascontiguousarray(A).reshape(N_CORES, NG * 128, BW)
    Bm = Bm.reshape(N_CORES, NG, G, 80, 128).transpose(0, 1, 3, 2, 4)
    Bm = np.ascontiguousarray(Bm).reshape(N_CORES, NG * 80, BW)
    return A, Bm


def _build_x_panels(x):
    x16 = x.astype(np.float16)                       # [S, B, F]
    v = x16.reshape(NB, 128, B, N_CORES, NG, G)      # [i, m, b, c, g, j]
    v = v.transpose(3, 4, 1, 5, 0, 2)                # [c, g, m, j, i, b]
    return np.ascontiguousarray(v).reshape(N_CORES, NG * 128, XW)


def _unpack_out(outs):
    o = np.stack(outs)                               # [c, NG*128, XW]
    o = o.reshape(N_CORES, NG, 128, G, NB, B)        # [c, g, m, j, i, b]
    o = o.transpose(4, 2, 5, 0, 1, 3)                # [i, m, b, c, g, j]
    return np.ascontiguousarray(o).reshape(S, B, F).astype(np.float32)


def _make_in_maps(x, weight):
    xp = _build_x_panels(np.asarray(x, dtype=np.float32))
    A, Bm = _build_bands(np.asarray(weight, dtype=np.float32))
    return [{"x": xp[c], "bandA": A[c], "bandB": Bm[c]}
            for c in range(N_CORES)]


def kernel(x, weight):
    global _compiled
    if _compiled is None:
        _compiled = _build_program()
    in_maps = _make_in_maps(x, weight)
    res = run_bass_kernel_spmd(_compiled, in_maps, list(range(N_CORES)))
    outs = [np.asarray(res.results[c]["out"]) for c in range(N_CORES)]
    return _unpack_out(outs)
